# revision 4
# baseline (speedup 1.0000x reference)
"""Edge-decoder (GNN link prediction) kernel for 8 Trainium2 NeuronCores.

Computes logits[e] = sum_d x[src[e], d] * x[tar[e], d] for 640K edges
(pos then neg), node table x [100000, 128] f32.

Strategy: edges sharded contiguously across 8 cores (80000/core), node
table replicated in bf16 (host-converted; rel-err budget 2e-2 leaves
a wide margin). All row fetches use the Pool-engine custom bulk gather
(InstDMAGatherAnt via gpsimd.dma_gather, mlp ucode library), thousands
of 256B rows per instruction, spread over the 4 SWDGE queues. Measured
pitfalls baked into this design:
  - one SWDGE queue serializes the gather stream (~7x slower) -> round
    robin over queue_num 0..3 with deep buffering.
  - dma_gather indices are int16, so a gather addresses only 32768 rows
    past its base. A register-offset (values_load) base and sliced or
    smaller-than-64K-row source APs all fall off the ucode fast path
    (5-15x slower). Only a full-tensor AP of the proven [100000, 128]
    shape is reliably fast, so the host passes FOUR rolled copies of
    the table (xk = roll(x, -32768k)) and every gather reads rows
    [0, 32768) of one of them with a static base.
  - edges are bucketed by src>>15 (4 buckets -> 2/2/2/1 supergroups),
    so a supergroup's src gathers live in one bucket; slots inside a
    supergroup are grouped by tar>>15 with capacity padding (-1 index
    tails are skipped by the ucode; true counts via num_idxs_reg).
  - natural (random) index order is kept: it spreads reads across HBM
    channels (measured faster than sorted).
Gathered tiles are [128, blocks, 128] bf16 (row i -> partition i%128,
block i//128). DVE multiplies src*tar into a scratch (tensor_tensor
runs 2 elem/cyc for bf16; tensor_reduce only 1) and a 2-level
tensor_tensor halving tree + short reduce produce f32 logits.
"""

import numpy as np

N_NODES = 100000
D = 128
E_TOTAL = 640000
N_CORES = 8
P = 128
E_CORE = E_TOTAL // N_CORES  # 80000
RNG_ROWS = 32768
NB = 4  # src buckets / tar groups
SGS_PER_BUCKET = [2, 2, 2, 1]
NSG = sum(SGS_PER_BUCKET)  # 7
SG_BUCKET = [0, 0, 1, 1, 2, 2, 3]  # src bucket of each supergroup
CAPS_BIG = [4736, 4736, 4736, 384]  # slot caps per tar group, big-bucket sg
CAPS_SMALL = [640, 640, 640, 128]
SG_CAPS = [CAPS_BIG] * 6 + [CAPS_SMALL]
SG_SLOTS = [sum(c) for c in SG_CAPS]  # 14592 x6, 2048
SG_BLKS = [s // P for s in SG_SLOTS]  # 114 x6, 16
BLK_MAX = max(SG_BLKS)
SG_BLK_OFF = np.cumsum([0] + SG_BLKS).tolist()  # into logits free dim
TOT_BLKS = SG_BLK_OFF[-1]  # 700
SG_IDX_COLS = [2 * s // 16 for s in SG_SLOTS]  # 1824 x6, 256
SG_COL_OFF = np.cumsum([0] + SG_IDX_COLS).tolist()
TOT_COLS = SG_COL_OFF[-1]  # 11200

_cached = {}


def build(reps=1):
    import concourse.bacc as bacc
    import concourse.bass as bass
    import concourse.mybir as mybir
    from concourse.library_config import mlp

    nc = bacc.Bacc(
        "TRN2",
        target_bir_lowering=False,
        debug=False,
        num_devices=N_CORES,
        num_swdge_queues=4,
    )
    xs = [
        nc.dram_tensor(f"x{k}", [N_NODES, D], mybir.dt.bfloat16, kind="ExternalInput")
        for k in range(NB)
    ]
    idx = nc.dram_tensor("idx", [P, TOT_COLS], mybir.dt.int16, kind="ExternalInput")
    meta = nc.dram_tensor("meta", [1, 32], mybir.dt.int32, kind="ExternalInput")
    logits = nc.dram_tensor(
        "logits", [P, TOT_BLKS], mybir.dt.float32, kind="ExternalOutput"
    )

    with (
        nc.Block() as block,
        nc.sbuf_tensor("idx_sb", [P, TOT_COLS], mybir.dt.int16) as idx_sb,
        nc.sbuf_tensor("meta_sb", [1, 32], mybir.dt.int32) as meta_sb,
        nc.sbuf_tensor("S", [P, 2, BLK_MAX, D], mybir.dt.bfloat16) as S,
        nc.sbuf_tensor("T", [P, 2, BLK_MAX, D], mybir.dt.bfloat16) as T,
        nc.sbuf_tensor("pr", [P, BLK_MAX, D], mybir.dt.bfloat16) as pr,
        nc.sbuf_tensor("s64", [P, BLK_MAX, 64], mybir.dt.bfloat16) as s64,
        nc.sbuf_tensor("s32", [P, BLK_MAX, 32], mybir.dt.bfloat16) as s32,
        nc.sbuf_tensor("lg", [P, 2, BLK_MAX], mybir.dt.float32) as lg,
        nc.semaphore("io") as io,
        nc.semaphore("g") as g,
        nc.semaphore("mu") as mu,
        nc.semaphore("dv") as dv,
        nc.semaphore("st") as st,
    ):
        n_gs = reps * NSG

        @block.sync
        def _(sync):
            sync.dma_start(idx_sb[:], idx[:]).then_inc(io, 16)
            sync.dma_start(meta_sb[:], meta[:]).then_inc(io, 16)

        @block.gpsimd
        def _(gpsimd: bass.BassGpSimd):
            gpsimd.load_library(mlp)
            gpsimd.wait_ge(io, 32)
            cnt_regs = [
                [
                    nc.values_load(
                        meta_sb[0:1, s * NB + m : s * NB + m + 1],
                        engines=(mybir.EngineType.Pool,),
                        min_val=0,
                        max_val=SG_CAPS[s][m],
                        skip_runtime_bounds_check=True,
                    )
                    for m in range(NB)
                ]
                for s in range(NSG)
            ]
            q = 0
            for gs in range(n_gs):
                s, b = gs % NSG, gs % 2
                caps = SG_CAPS[s]
                if gs >= 2:
                    # mult of gs-2 done -> S[b]/T[b] free
                    gpsimd.wait_ge(mu, gs - 1)
                col0 = SG_COL_OFF[s]
                blk0 = 0
                for m in range(NB):
                    cblk = caps[m] // P
                    ccols = caps[m] // 16
                    gpsimd.dma_gather(
                        S[:, b, blk0 : blk0 + cblk, :],
                        xs[SG_BUCKET[s]][:, :],
                        idx_sb[:, col0 : col0 + ccols],
                        caps[m],
                        cnt_regs[s][m],
                        D,
                        single_packet=False,
                        queue_num=q % 4,
                    ).then_inc(g, 16)
                    q += 1
                    gpsimd.dma_gather(
                        T[:, b, blk0 : blk0 + cblk, :],
                        xs[m][:, :],
                        idx_sb[:, col0 + ccols : col0 + 2 * ccols],
                        caps[m],
                        cnt_regs[s][m],
                        D,
                        single_packet=False,
                        queue_num=q % 4,
                    ).then_inc(g, 16)
                    q += 1
                    col0 += 2 * ccols
                    blk0 += cblk
            gpsimd.wait_ge(g, 16 * 8 * n_gs)

        @block.vector
        def _(vector):
            import concourse.mybir as mybir_

            add = mybir_.AluOpType.add
            for gs in range(n_gs):
                s, b = gs % NSG, gs % 2
                nb = SG_BLKS[s]
                vector.wait_ge(g, 16 * 8 * (gs + 1))
                vector.tensor_tensor(
                    out=pr[:, :nb, :],
                    in0=S[:, b, :nb, :],
                    in1=T[:, b, :nb, :],
                    op=mybir_.AluOpType.mult,
                ).then_inc(mu, 1)
                vector.tensor_tensor(
                    out=s64[:, :nb, :],
                    in0=pr[:, :nb, 0:64],
                    in1=pr[:, :nb, 64:128],
                    op=add,
                )
                vector.tensor_tensor(
                    out=s32[:, :nb, :],
                    in0=s64[:, :nb, 0:32],
                    in1=s64[:, :nb, 32:64],
                    op=add,
                )
                if gs >= 2:
                    # store of gs-2 done -> lg[b] free
                    vector.wait_ge(st, 16 * (gs - 1))
                vector.tensor_reduce(
                    out=lg[:, b, :nb],
                    in_=s32[:, :nb, :],
                    axis=mybir_.AxisListType.X,
                    op=add,
                ).then_inc(dv, 1)

        @block.scalar
        def _(scalar):
            for gs in range(n_gs):
                s, b = gs % NSG, gs % 2
                nb = SG_BLKS[s]
                scalar.wait_ge(dv, gs + 1)
                scalar.dma_start(
                    logits[:, SG_BLK_OFF[s] : SG_BLK_OFF[s] + nb], lg[:, b, :nb]
                ).then_inc(st, 16)
            scalar.wait_ge(st, 16 * n_gs)

    nc.compile()
    return nc


def _get_nc():
    if "nc" not in _cached:
        _cached["nc"] = build()
    return _cached["nc"]


def host_prepare(x, src, tar):
    """Per-core packing. Returns (in_maps, unpack)."""
    import ml_dtypes

    xb = np.asarray(x, np.float32).astype(ml_dtypes.bfloat16)
    xrolls = [np.ascontiguousarray(np.roll(xb, -k * RNG_ROWS, axis=0)) for k in range(NB)]
    in_maps, unpacks = [], []
    for c in range(N_CORES):
        s_all = src[c * E_CORE : (c + 1) * E_CORE].astype(np.int64)
        t_all = tar[c * E_CORE : (c + 1) * E_CORE].astype(np.int64)
        idx_cols = []
        counts = np.zeros(32, np.int32)
        slot_of = np.empty(E_CORE, np.int64)  # local edge -> blk128-based flat slot
        s_bkt = s_all >> 15
        sg_i = 0
        for k in range(NB):
            in_b = np.where(s_bkt == k)[0]
            nsg = SGS_PER_BUCKET[k]
            parts = np.array_split(in_b, nsg)
            for part in parts:
                s = sg_i
                caps = SG_CAPS[s]
                ts_p = t_all[part]
                gof = ts_p >> 15
                blk_base = SG_BLK_OFF[s]
                goff = 0
                for m in range(NB):
                    im = np.where(gof == m)[0]
                    cnt = len(im)
                    assert 0 < cnt <= caps[m], (c, s, m, cnt)
                    counts[s * NB + m] = cnt
                    eids = part[im]
                    slot_of[eids] = (blk_base + goff // P) * P + np.arange(cnt)
                    sl = np.full(caps[m], -1, np.int16)
                    sl[:cnt] = (s_all[eids] - (k << 15)).astype(np.int16)
                    tl = np.full(caps[m], -1, np.int16)
                    tl[:cnt] = (ts_p[im] - (m << 15)).astype(np.int16)
                    idx_cols.append(sl.reshape(-1, 16).T)
                    idx_cols.append(tl.reshape(-1, 16).T)
                    goff += caps[m]
                sg_i += 1
        blob = np.concatenate(idx_cols, axis=1)
        assert blob.shape == (16, TOT_COLS), blob.shape
        in_map = {f"x{k}": xrolls[k] for k in range(NB)}
        in_map["idx"] = np.ascontiguousarray(np.tile(blob, (8, 1)))
        in_map["meta"] = counts.reshape(1, 32)
        in_maps.append(in_map)
        unpacks.append(slot_of)

    def unpack(results):
        out = np.empty((E_TOTAL, 1), np.float32)
        for c in range(N_CORES):
            lgv = results[c]["logits"]  # [P, TOT_BLKS]
            flat = lgv.T.reshape(-1)  # blk*128 + p
            out[c * E_CORE : (c + 1) * E_CORE, 0] = flat[unpacks[c]]
        return out

    return in_maps, unpack


def kernel(x, pos_edge_index, neg_edge_index):
    from concourse.bass_utils import run_bass_kernel_spmd

    src = np.concatenate(
        [np.asarray(pos_edge_index[0]), np.asarray(neg_edge_index[0])]
    ).astype(np.int32)
    tar = np.concatenate(
        [np.asarray(pos_edge_index[1]), np.asarray(neg_edge_index[1])]
    ).astype(np.int32)

    in_maps, unpack = host_prepare(x, src, tar)
    nc = _get_nc()
    res = run_bass_kernel_spmd(nc, in_maps, core_ids=list(range(N_CORES)))
    return unpack(res.results)


# revision 9
# speedup vs baseline: 1.9294x; 1.9294x over previous
"""Edge-decoder (GNN link prediction) kernel for 8 Trainium2 NeuronCores.

Computes logits[e] = sum_d x[src[e], d] * x[tar[e], d] for 640K edges
(pos then neg), node table x [100000, 128] f32.

Strategy: edges sharded contiguously across 8 cores (80000/core), node
table replicated in bf16 (host-converted; rel-err budget 2e-2 leaves
a wide margin). All row fetches use the Pool-engine custom bulk gather
(InstDMAGatherAnt via gpsimd.dma_gather, mlp ucode library), thousands
of 256B rows per instruction, spread over the 4 SWDGE queues. Measured
pitfalls baked into this design:
  - one SWDGE queue serializes the gather stream (~7x slower) -> round
    robin over queue_num 0..3 with deep buffering.
  - dma_gather indices are int16, so a gather addresses only 32768 rows
    past its base. A register-offset (values_load) base and sliced or
    smaller-than-64K-row source APs all fall off the ucode fast path
    (5-15x slower). Only a full-tensor AP of the proven [100000, 128]
    shape is reliably fast, so the host passes FOUR rolled copies of
    the table (xk = roll(x, -32768k)) and every gather reads rows
    [0, 32768) of one of them with a static base.
  - edges are bucketed by src>>15 (4 buckets -> 2/2/2/1 supergroups),
    so a supergroup's src gathers live in one bucket; slots inside a
    supergroup are grouped by tar>>15 with capacity padding (-1 index
    tails are skipped by the ucode; true counts via num_idxs_reg).
  - natural (random) index order is kept: it spreads reads across HBM
    channels (measured faster than sorted).
Gathered tiles are [128, blocks, 128] bf16 (row i -> partition i%128,
block i//128). DVE multiplies src*tar into a scratch (tensor_tensor
runs 2 elem/cyc for bf16; tensor_reduce only 1) and a 2-level
tensor_tensor halving tree + short reduce produce f32 logits.
"""

import numpy as np

N_NODES = 100000
D = 128
E_TOTAL = 640000
N_CORES = 8
P = 128
E_CORE = E_TOTAL // N_CORES  # 80000
RNG_ROWS = 32768
NB = 4  # src buckets / tar groups
SGS_PER_BUCKET = [2, 2, 2, 1]
NSG = sum(SGS_PER_BUCKET)  # 7
SG_BUCKET = [0, 0, 1, 1, 2, 2, 3]  # src bucket of each supergroup
CAPS_BIG = [4736, 4736, 4736, 384]  # slot caps per tar group, big-bucket sg
CAPS_SMALL = [640, 640, 640, 128]
SG_CAPS = [CAPS_BIG] * 6 + [CAPS_SMALL]
SG_SLOTS = [sum(c) for c in SG_CAPS]  # 14592 x6, 2048
SG_BLKS = [s // P for s in SG_SLOTS]  # 114 x6, 16
BLK_MAX = max(SG_BLKS)
SG_BLK_OFF = np.cumsum([0] + SG_BLKS).tolist()  # into logits free dim
TOT_BLKS = SG_BLK_OFF[-1]  # 700
SG_IDX_COLS = [2 * s // 16 for s in SG_SLOTS]  # 1824 x6, 256
SG_COL_OFF = np.cumsum([0] + SG_IDX_COLS).tolist()
TOT_COLS = SG_COL_OFF[-1]  # 11200

_cached = {}


def build(reps=1):
    import concourse.bacc as bacc
    import concourse.bass as bass
    import concourse.mybir as mybir
    from concourse.library_config import mlp

    nc = bacc.Bacc(
        "TRN2",
        target_bir_lowering=False,
        debug=False,
        num_devices=N_CORES,
        num_swdge_queues=4,
    )
    xs = [
        nc.dram_tensor(f"x{k}", [N_NODES, D], mybir.dt.bfloat16, kind="ExternalInput")
        for k in range(NB)
    ]
    idx = nc.dram_tensor("idx", [P, TOT_COLS], mybir.dt.int16, kind="ExternalInput")
    meta = nc.dram_tensor("meta", [1, 32], mybir.dt.int32, kind="ExternalInput")
    logits = nc.dram_tensor(
        "logits", [P, TOT_BLKS], mybir.dt.float32, kind="ExternalOutput"
    )

    with (
        nc.Block() as block,
        nc.sbuf_tensor("idx_sb", [P, TOT_COLS], mybir.dt.int16) as idx_sb,
        nc.sbuf_tensor("meta_sb", [1, 32], mybir.dt.int32) as meta_sb,
        nc.sbuf_tensor("S", [P, 2, BLK_MAX, D], mybir.dt.bfloat16) as S,
        nc.sbuf_tensor("T", [P, 2, BLK_MAX, D], mybir.dt.bfloat16) as T,
        nc.sbuf_tensor("pr", [P, BLK_MAX, D], mybir.dt.bfloat16) as pr,
        nc.sbuf_tensor("s64", [P, BLK_MAX, 64], mybir.dt.bfloat16) as s64,
        nc.sbuf_tensor("s32", [P, BLK_MAX, 32], mybir.dt.bfloat16) as s32,
        nc.sbuf_tensor("lg", [P, 2, BLK_MAX], mybir.dt.float32) as lg,
        nc.semaphore("io") as io,
        nc.semaphore("g0") as g0,
        nc.semaphore("g1") as g1,
        nc.semaphore("mu") as mu,
        nc.semaphore("dv") as dv,
        nc.semaphore("st") as st,
    ):
        gsems = [g0, g1]
        n_gs = reps * NSG

        @block.sync
        def _(sync):
            sync.dma_start(idx_sb[:], idx[:]).then_inc(io, 16)
            sync.dma_start(meta_sb[:], meta[:]).then_inc(io, 16)

        @block.gpsimd
        def _(gpsimd: bass.BassGpSimd):
            gpsimd.load_library(mlp)
            gpsimd.wait_ge(io, 32)
            cnt_regs = [
                [
                    nc.values_load(
                        meta_sb[0:1, s * NB + m : s * NB + m + 1],
                        engines=(mybir.EngineType.Pool,),
                        min_val=0,
                        max_val=SG_CAPS[s][m],
                        skip_runtime_bounds_check=True,
                    )
                    for m in range(NB)
                ]
                for s in range(NSG)
            ]
            q = 0
            for gs in range(n_gs):
                s, b = gs % NSG, gs % 2
                caps = SG_CAPS[s]
                if gs >= 2:
                    # mult of gs-2 done -> S[b]/T[b] free
                    gpsimd.wait_ge(mu, gs - 1)
                col0 = SG_COL_OFF[s]
                blk0 = 0
                for m in range(NB):
                    cblk = caps[m] // P
                    ccols = caps[m] // 16
                    gpsimd.dma_gather(
                        S[:, b, blk0 : blk0 + cblk, :],
                        xs[SG_BUCKET[s]][:, :],
                        idx_sb[:, col0 : col0 + ccols],
                        caps[m],
                        cnt_regs[s][m],
                        D,
                        single_packet=False,
                        queue_num=q % 4,
                    ).then_inc(gsems[b], 16)
                    q += 1
                    gpsimd.dma_gather(
                        T[:, b, blk0 : blk0 + cblk, :],
                        xs[m][:, :],
                        idx_sb[:, col0 + ccols : col0 + 2 * ccols],
                        caps[m],
                        cnt_regs[s][m],
                        D,
                        single_packet=False,
                        queue_num=q % 4,
                    ).then_inc(gsems[b], 16)
                    q += 1
                    col0 += 2 * ccols
                    blk0 += cblk
            for bb in range(2):
                rounds = (n_gs - bb + 1) // 2
                if rounds > 0:
                    gpsimd.wait_ge(gsems[bb], 16 * 8 * rounds)

        @block.vector
        def _(vector):
            import concourse.mybir as mybir_

            add = mybir_.AluOpType.add
            for gs in range(n_gs):
                s, b = gs % NSG, gs % 2
                nb = SG_BLKS[s]
                vector.wait_ge(gsems[b], 16 * 8 * (gs // 2 + 1))
                vector.tensor_tensor(
                    out=pr[:, :nb, :],
                    in0=S[:, b, :nb, :],
                    in1=T[:, b, :nb, :],
                    op=mybir_.AluOpType.mult,
                ).then_inc(mu, 1)
                vector.tensor_tensor(
                    out=s64[:, :nb, :],
                    in0=pr[:, :nb, 0:64],
                    in1=pr[:, :nb, 64:128],
                    op=add,
                )
                vector.tensor_tensor(
                    out=s32[:, :nb, :],
                    in0=s64[:, :nb, 0:32],
                    in1=s64[:, :nb, 32:64],
                    op=add,
                )
                if gs >= 2:
                    # store of gs-2 done -> lg[b] free
                    vector.wait_ge(st, 16 * (gs - 1))
                vector.tensor_reduce(
                    out=lg[:, b, :nb],
                    in_=s32[:, :nb, :],
                    axis=mybir_.AxisListType.X,
                    op=add,
                ).then_inc(dv, 1)

        @block.scalar
        def _(scalar):
            for gs in range(n_gs):
                s, b = gs % NSG, gs % 2
                nb = SG_BLKS[s]
                scalar.wait_ge(dv, gs + 1)
                scalar.dma_start(
                    logits[:, SG_BLK_OFF[s] : SG_BLK_OFF[s] + nb], lg[:, b, :nb]
                ).then_inc(st, 16)
            scalar.wait_ge(st, 16 * n_gs)

    nc.compile()
    return nc


def _get_nc():
    if "nc" not in _cached:
        _cached["nc"] = build()
    return _cached["nc"]


def host_prepare(x, src, tar):
    """Per-core packing. Returns (in_maps, unpack)."""
    import ml_dtypes

    xb = np.asarray(x, np.float32).astype(ml_dtypes.bfloat16)
    xrolls = [np.ascontiguousarray(np.roll(xb, -k * RNG_ROWS, axis=0)) for k in range(NB)]
    in_maps, unpacks = [], []
    for c in range(N_CORES):
        s_all = src[c * E_CORE : (c + 1) * E_CORE].astype(np.int64)
        t_all = tar[c * E_CORE : (c + 1) * E_CORE].astype(np.int64)
        idx_cols = []
        counts = np.zeros(32, np.int32)
        slot_of = np.empty(E_CORE, np.int64)  # local edge -> blk128-based flat slot
        s_bkt = s_all >> 15
        sg_i = 0
        for k in range(NB):
            in_b = np.where(s_bkt == k)[0]
            nsg = SGS_PER_BUCKET[k]
            parts = np.array_split(in_b, nsg)
            for part in parts:
                s = sg_i
                caps = SG_CAPS[s]
                ts_p = t_all[part]
                gof = ts_p >> 15
                blk_base = SG_BLK_OFF[s]
                goff = 0
                for m in range(NB):
                    im = np.where(gof == m)[0]
                    cnt = len(im)
                    assert 0 < cnt <= caps[m], (c, s, m, cnt)
                    counts[s * NB + m] = cnt
                    eids = part[im]
                    slot_of[eids] = (blk_base + goff // P) * P + np.arange(cnt)
                    sl = np.full(caps[m], -1, np.int16)
                    sl[:cnt] = (s_all[eids] - (k << 15)).astype(np.int16)
                    tl = np.full(caps[m], -1, np.int16)
                    tl[:cnt] = (ts_p[im] - (m << 15)).astype(np.int16)
                    idx_cols.append(sl.reshape(-1, 16).T)
                    idx_cols.append(tl.reshape(-1, 16).T)
                    goff += caps[m]
                sg_i += 1
        blob = np.concatenate(idx_cols, axis=1)
        assert blob.shape == (16, TOT_COLS), blob.shape
        in_map = {f"x{k}": xrolls[k] for k in range(NB)}
        in_map["idx"] = np.ascontiguousarray(np.tile(blob, (8, 1)))
        in_map["meta"] = counts.reshape(1, 32)
        in_maps.append(in_map)
        unpacks.append(slot_of)

    def unpack(results):
        out = np.empty((E_TOTAL, 1), np.float32)
        for c in range(N_CORES):
            lgv = results[c]["logits"]  # [P, TOT_BLKS]
            flat = lgv.T.reshape(-1)  # blk*128 + p
            out[c * E_CORE : (c + 1) * E_CORE, 0] = flat[unpacks[c]]
        return out

    return in_maps, unpack


def kernel(x, pos_edge_index, neg_edge_index):
    from concourse.bass_utils import run_bass_kernel_spmd

    src = np.concatenate(
        [np.asarray(pos_edge_index[0]), np.asarray(neg_edge_index[0])]
    ).astype(np.int32)
    tar = np.concatenate(
        [np.asarray(pos_edge_index[1]), np.asarray(neg_edge_index[1])]
    ).astype(np.int32)

    in_maps, unpack = host_prepare(x, src, tar)
    nc = _get_nc()
    res = run_bass_kernel_spmd(nc, in_maps, core_ids=list(range(N_CORES)))
    return unpack(res.results)
